# revision 12
# baseline (speedup 1.0000x reference)
"""Causal multi-head self-attention with RoPE on 8 Trainium2 NeuronCores.

Sharding: data parallel over batch (2) x tensor parallel over heads (4 groups
of 4 heads).  Core c handles batch b = c // 4, head group hg = c % 4.

Per-core dataflow (everything stays in "transposed" [feature, seq] layouts so
no on-device transposes are ever needed):
  QT = wqT.T @ xT   [256, 2048]   (fp32r matmuls, d-contraction on partitions)
  RoPE via a constant shuffle matmul: rot = QT*cosT + (S @ (QT*sinT))
  V  = xT.T @ wvT   [2048, 256] -> fp16, augmented with a ones column per head
  per head h:
    scores^T[ktile j] = Krot_h[:,128j:128j+128].T @ Qrot_h   (k on partitions)
    expP = exp(scores/8) fp16 ; diagonal 128x128 block *= triangular mask
    per 512-query chunk: psum[65,512] = sum_j V_aug_j.T @ expP_j  (fp16 matmul)
       row 64 is the softmax denominator (ones column of V_aug)
    A^T = psum[:64] * bcast(1/psum[64])  -> fp16
  AllGather (one per head index, groups [[0..3],[4..7]]) -> full A^T [1024,2048]
  outT = woT.T @ agT  [256, 2048] fp32  (fp16 matmuls, W_o rows pre-permuted)
Host: out[b, :, 256*hg:256*(hg+1)] = outT.T
"""

import numpy as np

import concourse.bass as bass
import concourse.mybir as mybir
import concourse.tile as tile
from concourse import bacc
from concourse.bass_utils import run_bass_kernel_spmd

F32 = mybir.dt.float32
F32R = mybir.dt.float32r
F16 = mybir.dt.float16

B, S, D, H, DH = 2, 2048, 1024, 16, 64
ROPE_THETA = 10000.0
NCORE = 8
HPG = 4          # heads per group (per core)
P = 128
NKT = S // P     # 16 k-tiles
NQC = S // 512   # 4 query chunks

# expP storage: k-tile j's columns start at global q = 512*(j//4); width below.
_W = [S - P * j for j in range(NKT)]
_OFF = np.concatenate([[0], np.cumsum(_W)]).astype(int)
EXP_TOT = int(_OFF[-1])  # 17408 columns of fp16 -> 34KB/partition


def build_program():
    nc = bacc.Bacc(
        "TRN2", target_bir_lowering=False, debug=False, num_devices=NCORE
    )

    xT = nc.dram_tensor("xT", [D, S], F16, kind="ExternalInput")
    wqT = nc.dram_tensor("wqT", [D, 256], F16, kind="ExternalInput")
    wkT = nc.dram_tensor("wkT", [D, 256], F16, kind="ExternalInput")
    wvT = nc.dram_tensor("wvT", [D, 256], F16, kind="ExternalInput")
    woT = nc.dram_tensor("woT", [D, 256], F16, kind="ExternalInput")
    cosT = nc.dram_tensor("cosT", [P, S], F32, kind="ExternalInput")
    sinT = nc.dram_tensor("sinT", [P, S], F32, kind="ExternalInput")
    ST = nc.dram_tensor("ST", [P, P], F32R, kind="ExternalInput")
    trimask = nc.dram_tensor("trimask", [P, P], F16, kind="ExternalInput")

    ag_in = nc.dram_tensor("ag_in", [HPG, 64, S], F16)
    ag_out = nc.dram_tensor("ag_out", [HPG, 256, S], F16)
    outT = nc.dram_tensor("outT", [256, S], F32, kind="ExternalOutput")

    groups = [[0, 1, 2, 3], [4, 5, 6, 7]]

    with tile.TileContext(nc) as tc:
        with (
            tc.tile_pool(name="const", bufs=1) as cpool,
            tc.tile_pool(name="qkv", bufs=1) as qkv,
            tc.tile_pool(name="psum", bufs=1, space="PSUM") as psum,
            tc.tile_pool(name="agp", bufs=4) as agp,
        ):
            tri_sb = cpool.tile([P, P], F16, tag="tri")
            wo_sb = cpool.tile([P, 8, 256], F16, tag="wo")
            qrot = qkv.tile([P, 2, S], F32R, tag="qrot")
            krot = qkv.tile([P, 2, S], F32R, tag="krot")
            v_sb = qkv.tile([P, NKT, HPG, DH + 1], F16, tag="v")
            at_sb = qkv.tile([P, 2, S], F16, tag="at")
            acc_sb = qkv.tile([P, 2, S], F32, tag="acc")

            # ---------------- phase 1: projections + rope -----------------
            with tc.tile_pool(name="p1", bufs=1) as p1:
                wq_sb = p1.tile([P, 8, 256], F16, tag="wq")
                xt_sb = p1.tile([P, 8, S], F16, tag="xt")
                wk_sb = p1.tile([P, 8, 256], F16, tag="wk")
                wv_sb = p1.tile([P, 8, 256], F16, tag="wv")
                cos_sb = p1.tile([P, S], F32, tag="cos")
                sin_sb = p1.tile([P, S], F32, tag="sin")
                st_sb = p1.tile([P, P], F32R, tag="st")

                # bulk operands on the sync (HW DGE) queue, wq + x first so
                # the first projection matmuls can start ASAP
                nc.sync.dma_start(
                    out=wq_sb[:], in_=wqT.rearrange("(n p) m -> p n m", p=P)
                )
                for dt in range(8):
                    nc.sync.dma_start(
                        out=xt_sb[:, dt, :], in_=xT[P * dt:P * (dt + 1), :]
                    )
                nc.sync.dma_start(
                    out=wk_sb[:], in_=wkT.rearrange("(n p) m -> p n m", p=P)
                )
                nc.sync.dma_start(
                    out=wv_sb[:], in_=wvT.rearrange("(n p) m -> p n m", p=P)
                )
                # constants on the gpsimd (SW DGE) queue in parallel
                nc.gpsimd.dma_start(out=sin_sb[:], in_=sinT[:, :])
                nc.gpsimd.dma_start(out=cos_sb[:], in_=cosT[:, :])
                nc.gpsimd.dma_start(out=st_sb[:], in_=ST[:, :])
                nc.gpsimd.dma_start(out=tri_sb[:], in_=trimask[:, :])

                nc.gpsimd.dma_start(
                    out=wo_sb[:], in_=woT.rearrange("(n p) m -> p n m", p=P)
                )

                nc.vector.memset(v_sb[:, :, :, DH:DH + 1], 1.0)

                # Q/K projections + rope
                for w_sb, rot in ((wq_sb, qrot), (wk_sb, krot)):
                    for mt in range(2):
                        for sc in range(4):
                            ssl = bass.ts(sc, 512)
                            pp = psum.tile([P, 512], F32, tag="proj", bufs=2)
                            for dt in range(8):
                                nc.tensor.matmul(
                                    pp[:],
                                    w_sb[:, dt, P * mt:P * (mt + 1)],
                                    xt_sb[:, dt, ssl],
                                    start=(dt == 0),
                                    stop=(dt == 7),
                                )
                            t_s = p1.tile([P, 512], F32R, tag="ts", bufs=3)
                            nc.vector.tensor_tensor(
                                out=t_s[:], in0=pp[:], in1=sin_sb[:, ssl],
                                op=mybir.AluOpType.mult,
                            )
                            sh = psum.tile([P, 512], F32, tag="shuf", bufs=1)
                            nc.tensor.matmul(
                                sh[:], st_sb[:], t_s[:], start=True, stop=True
                            )
                            nc.vector.tensor_tensor(
                                out=rot[:, mt, ssl], in0=pp[:],
                                in1=cos_sb[:, ssl], op=mybir.AluOpType.mult,
                            )
                            nc.vector.tensor_tensor(
                                out=rot[:, mt, ssl], in0=rot[:, mt, ssl],
                                in1=sh[:], op=mybir.AluOpType.add,
                            )

                # V projection -> fp16 V_aug (dense PE bridge into attention)
                for st in range(NKT):
                    vp = psum.tile([P, 256], F32, tag="vproj", bufs=1)
                    for dt in range(8):
                        nc.tensor.matmul(
                            vp[:],
                            xt_sb[:, dt, P * st:P * (st + 1)],
                            wv_sb[:, dt, :],
                            start=(dt == 0),
                            stop=(dt == 7),
                        )
                    nc.vector.tensor_copy(
                        out=v_sb[:, st, :, 0:DH],
                        in_=vp.rearrange("p (h d) -> p h d", h=HPG),
                    )

            # ---------------- phase 2+3: attention + output projection -----
            with tc.tile_pool(name="atmp", bufs=4) as atmp:
                ep = qkv.tile([P, EXP_TOT], F16, tag="expp")
                piece_ctr = 0
                for h in range(HPG):
                    th, bs = h // 2, 64 * (h % 2)
                    qh = qrot[bs:bs + 64, th, :]
                    kh = krot[bs:bs + 64, th, :]
                    for j in range(NKT):
                        c0 = j // 4
                        r = j % 4
                        off = int(_OFF[j])
                        cs = list(range(c0, 4))
                        grps = [cs[i:i + 2] for i in range(0, len(cs), 2)]
                        for grp in grps:
                            ca = grp[0]
                            tag = "sA" if piece_ctr % 2 == 0 else "sB"
                            piece_ctr += 1
                            sp = psum.tile(
                                [P, 1024], F32, tag=tag, bufs=1, name="sp"
                            )
                            for c in grp:
                                loc = 512 * (c - ca)
                                if c == c0:
                                    nc.tensor.matmul(
                                        sp[:, loc + 128 * r:loc + 512],
                                        kh[:, P * j:P * (j + 1)],
                                        qh[:, 512 * c + 128 * r:512 * (c + 1)],
                                        start=True, stop=True,
                                    )
                                else:
                                    nc.tensor.matmul(
                                        sp[:, loc:loc + 512],
                                        kh[:, P * j:P * (j + 1)],
                                        qh[:, 512 * c:512 * (c + 1)],
                                        start=True, stop=True,
                                    )
                            # exp, skipping the sub-diagonal garbage columns
                            ls = 128 * r if ca == c0 else 0
                            qstart = 512 * ca + ls
                            w = 512 * (grp[-1] + 1) - qstart
                            eo = off + qstart - 128 * j
                            nc.scalar.activation(
                                out=ep[:, eo:eo + w],
                                in_=sp[:, ls:ls + w],
                                func=mybir.ActivationFunctionType.Exp,
                                scale=0.125,
                            )
                        nc.gpsimd.tensor_tensor(
                            out=ep[:, off:off + P], in0=ep[:, off:off + P],
                            in1=tri_sb[:], op=mybir.AluOpType.mult,
                        )
                    # P.T @ V per query chunk (+ softmax denominator row)
                    for c in range(NQC):
                        pv = psum.tile(
                            [P, 512], F32, tag="proj", bufs=2, name="pv"
                        )
                        last_j = 4 * c + 3
                        for j in range(last_j + 1):
                            off = int(_OFF[j])
                            if j // 4 == c:
                                rr = j % 4
                                n = 512 - 128 * rr
                                nc.tensor.matmul(
                                    pv[0:DH + 1, 128 * rr:512],
                                    v_sb[:, j, h, :],
                                    ep[:, off:off + n],
                                    start=(j == 0), stop=(j == last_j),
                                )
                            else:
                                st_col = off + 512 * c - 128 * j
                                nc.tensor.matmul(
                                    pv[0:DH + 1, :],
                                    v_sb[:, j, h, :],
                                    ep[:, st_col:st_col + 512],
                                    start=(j == 0), stop=(j == last_j),
                                )
                        den = atmp.tile([1, 512], F32, tag="den")
                        nc.vector.tensor_copy(out=den[:], in_=pv[DH:DH + 1, :])
                        recip = atmp.tile([1, 512], F32, tag="recip")
                        nc.vector.reciprocal_approx_fast(out=recip[:], in_=den[:])
                        bcast = atmp.tile([64, 512], F32, tag="bcast")
                        nc.gpsimd.partition_broadcast(bcast[:], recip[:])
                        nc.vector.tensor_tensor(
                            out=at_sb[bs:bs + 64, th, 512 * c:512 * (c + 1)],
                            in0=pv[0:DH, :], in1=bcast[:],
                            op=mybir.AluOpType.mult,
                        )
                    # all-gather this head, then fold its slice of the output
                    # projection into the accumulator (runs under later heads)
                    nc.sync.dma_start(
                        out=ag_in[h, :, :], in_=at_sb[bs:bs + 64, th, :]
                    )
                    nc.gpsimd.collective_compute(
                        "AllGather",
                        mybir.AluOpType.bypass,
                        replica_groups=groups,
                        ins=[ag_in[h, :, :]],
                        outs=[ag_out[h, :, :]],
                    )
                    for sc in range(4):
                        ssl = bass.ts(sc, 512)
                        rhs = [None, None]
                        for ct in range(2):
                            rhs[ct] = agp.tile(
                                [P, 512], F16, tag="ag", name="rhs"
                            )
                            nc.sync.dma_start(
                                out=rhs[ct][:],
                                in_=ag_out[h, P * ct:P * (ct + 1), ssl],
                            )
                        for ot in range(2):
                            po = psum.tile(
                                [P, 512], F32, tag="shuf", bufs=1, name="po"
                            )
                            for ct in range(2):
                                nc.tensor.matmul(
                                    po[:],
                                    wo_sb[:, 2 * h + ct, P * ot:P * (ot + 1)],
                                    rhs[ct][:],
                                    start=(ct == 0), stop=(ct == 1),
                                )
                            if h == 0:
                                nc.vector.tensor_copy(
                                    out=acc_sb[:, ot, ssl], in_=po[:]
                                )
                            else:
                                nc.vector.tensor_tensor(
                                    out=acc_sb[:, ot, ssl],
                                    in0=acc_sb[:, ot, ssl], in1=po[:],
                                    op=mybir.AluOpType.add,
                                )
                for ot in range(2):
                    nc.sync.dma_start(
                        out=outT[P * ot:P * (ot + 1), :], in_=acc_sb[:, ot, :]
                    )

    nc.compile()
    return nc


_PROGRAM = None


def _get_program():
    global _PROGRAM
    if _PROGRAM is None:
        _PROGRAM = build_program()
    return _PROGRAM


def _host_consts(token_positions):
    pos = np.asarray(token_positions, dtype=np.float32)
    inv = (
        ROPE_THETA ** (-np.arange(0, DH, 2, dtype=np.float32) / DH)
    ).astype(np.float32)
    ang = pos[:, None] * inv[None, :]  # [S, 32]
    cos, sin = np.cos(ang), np.sin(ang)
    rows = (np.arange(P) % DH) // 2
    cosT = np.ascontiguousarray(cos.T[rows]).astype(np.float32)
    sinT = np.ascontiguousarray(sin.T[rows]).astype(np.float32)
    Smat = np.zeros((P, P), dtype=np.float32)
    idx = np.arange(0, P, 2)
    Smat[idx, idx + 1] = -1.0
    Smat[idx + 1, idx] = 1.0
    ST = np.ascontiguousarray(Smat.T)
    tri = (np.arange(P)[None, :] >= np.arange(P)[:, None]).astype(np.float16)
    return cosT, sinT, ST, tri


def _make_in_maps(x, W_q, W_k, W_v, W_o, token_positions):
    cosT, sinT, ST, tri = _host_consts(token_positions)
    x = np.asarray(x, dtype=np.float32)
    maps = []
    for core in range(NCORE):
        b, hg = core // 4, core % 4
        hsl = slice(256 * hg, 256 * (hg + 1))
        # W_o rows for this core's output columns, input dims permuted to
        # (head, group, dh) order to match the per-head AllGather layout.
        wo = np.asarray(W_o, dtype=np.float32)[hsl]          # [256, 1024]
        wo_p = wo.reshape(256, 4, HPG, DH).transpose(0, 2, 1, 3).reshape(256, D)
        maps.append(
            {
                "xT": np.ascontiguousarray(x[b].T).astype(np.float16),
                "wqT": np.ascontiguousarray(np.asarray(W_q, np.float32)[hsl].T).astype(np.float16),
                "wkT": np.ascontiguousarray(np.asarray(W_k, np.float32)[hsl].T).astype(np.float16),
                "wvT": np.ascontiguousarray(np.asarray(W_v, np.float32)[hsl].T).astype(np.float16),
                "woT": np.ascontiguousarray(wo_p.T).astype(np.float16),
                "cosT": cosT,
                "sinT": sinT,
                "ST": ST,
                "trimask": tri,
            }
        )
    return maps


def _assemble(results):
    out = np.zeros((B, S, D), dtype=np.float32)
    for core in range(NCORE):
        b, hg = core // 4, core % 4
        out[b, :, 256 * hg:256 * (hg + 1)] = results[core]["outT"].T
    return out


def _run(in_maps, trace=False):
    nc = _get_program()
    tmpdir = None
    if trace:
        import tempfile

        tmpdir = tempfile.mkdtemp(prefix="ntff_", dir="/tmp")
    res = run_bass_kernel_spmd(
        nc, in_maps, list(range(NCORE)), trace=trace, tmpdir=tmpdir
    )
    return res


def kernel(x, W_q, W_k, W_v, W_o, token_positions):
    in_maps = _make_in_maps(x, W_q, W_k, W_v, W_o, token_positions)
    res = _run(in_maps)
    return _assemble(res.results)


def _install_profile_hook():
    """The agent image's antenv lacks axon_hooks; shim it so trace=True works."""
    import sys
    import types

    try:
        from antenv.axon_hooks import get_axon_ntff_profile_hook  # noqa: F401
        return
    except ImportError:
        pass
    import antenv
    from trn_agent_boot.trn_boot import _ntff_profile_via_ctypes

    mod = types.ModuleType("antenv.axon_hooks")
    _hook = {"h": None}
    mod.set_axon_ntff_profile_hook = lambda h: _hook.__setitem__("h", h)
    mod.get_axon_ntff_profile_hook = lambda: _hook["h"]
    sys.modules["antenv.axon_hooks"] = mod
    antenv.axon_hooks = mod
    mod.set_axon_ntff_profile_hook(
        _ntff_profile_via_ctypes("/opt/axon/libaxon_pjrt.so")
    )
    import concourse.bass_utils as bu

    bu.upload_artifacts = lambda d: f"file://{d}"


def kernel_traced(x, W_q, W_k, W_v, W_o, token_positions):
    """Returns (output, exec_time_ns, trace_path)."""
    _install_profile_hook()
    in_maps = _make_in_maps(x, W_q, W_k, W_v, W_o, token_positions)
    res = _run(in_maps, trace=True)
    trace_path = None
    if res.instructions_and_trace is not None:
        trace_path = res.instructions_and_trace[1]
    return _assemble(res.results), res.exec_time_ns, trace_path


# revision 13
# speedup vs baseline: 1.2704x; 1.2704x over previous
"""Causal multi-head self-attention with RoPE on 8 Trainium2 NeuronCores.

Sharding: data parallel over batch (2) x tensor parallel over heads (4 groups
of 4 heads).  Core c handles batch b = c // 4, head group hg = c % 4.

Per-core dataflow (everything stays in "transposed" [feature, seq] layouts so
no on-device transposes are ever needed):
  QT = wqT.T @ xT   [256, 2048]   (fp32r matmuls, d-contraction on partitions)
  RoPE via a constant shuffle matmul: rot = QT*cosT + (S @ (QT*sinT))
  V  = xT.T @ wvT   [2048, 256] -> fp16, augmented with a ones column per head
  per head h:
    scores^T[ktile j] = Krot_h[:,128j:128j+128].T @ Qrot_h   (k on partitions)
    expP = exp(scores/8) fp16 ; diagonal 128x128 block *= triangular mask
    per 512-query chunk: psum[65,512] = sum_j V_aug_j.T @ expP_j  (fp16 matmul)
       row 64 is the softmax denominator (ones column of V_aug)
    A^T = psum[:64] * bcast(1/psum[64])  -> fp16
  AllGather (one per head index, groups [[0..3],[4..7]]) -> full A^T [1024,2048]
  outT = woT.T @ agT  [256, 2048] fp32  (fp16 matmuls, W_o rows pre-permuted)
Host: out[b, :, 256*hg:256*(hg+1)] = outT.T
"""

import numpy as np

import concourse.bass as bass
import concourse.mybir as mybir
import concourse.tile as tile
from concourse import bacc
from concourse.bass_utils import run_bass_kernel_spmd

F32 = mybir.dt.float32
F32R = mybir.dt.float32r
F16 = mybir.dt.float16

B, S, D, H, DH = 2, 2048, 1024, 16, 64
ROPE_THETA = 10000.0
NCORE = 8
HPG = 4          # heads per group (per core)
P = 128
NKT = S // P     # 16 k-tiles
NQC = S // 512   # 4 query chunks

# expP storage: k-tile j's columns start at global q = 512*(j//4); width below.
_W = [S - P * j for j in range(NKT)]
_OFF = np.concatenate([[0], np.cumsum(_W)]).astype(int)
EXP_TOT = int(_OFF[-1])  # 17408 columns of fp16 -> 34KB/partition


def build_program():
    nc = bacc.Bacc(
        "TRN2", target_bir_lowering=False, debug=False, num_devices=NCORE
    )

    xT = nc.dram_tensor("xT", [D, S], F16, kind="ExternalInput")
    wqT = nc.dram_tensor("wqT", [D, 256], F16, kind="ExternalInput")
    wkT = nc.dram_tensor("wkT", [D, 256], F16, kind="ExternalInput")
    wvT = nc.dram_tensor("wvT", [D, 256], F16, kind="ExternalInput")
    woT = nc.dram_tensor("woT", [D, 256], F16, kind="ExternalInput")
    cosT = nc.dram_tensor("cosT", [P, S], F16, kind="ExternalInput")
    sinT = nc.dram_tensor("sinT", [P, S], F16, kind="ExternalInput")
    ST = nc.dram_tensor("ST", [P, P], F16, kind="ExternalInput")
    trimask = nc.dram_tensor("trimask", [P, P], F16, kind="ExternalInput")

    ag_in = nc.dram_tensor("ag_in", [HPG, 64, S], F16)
    ag_out = nc.dram_tensor("ag_out", [HPG, 256, S], F16)
    outT = nc.dram_tensor("outT", [256, S], F32, kind="ExternalOutput")

    groups = [[0, 1, 2, 3], [4, 5, 6, 7]]

    with tile.TileContext(nc) as tc:
        with (
            tc.tile_pool(name="const", bufs=1) as cpool,
            tc.tile_pool(name="qkv", bufs=1) as qkv,
            tc.tile_pool(name="psum", bufs=1, space="PSUM") as psum,
            tc.tile_pool(name="agp", bufs=4) as agp,
        ):
            tri_sb = cpool.tile([P, P], F16, tag="tri")
            wo_sb = cpool.tile([P, 8, 256], F16, tag="wo")
            qrot = qkv.tile([P, 2, S], F16, tag="qrot")
            krot = qkv.tile([P, 2, S], F16, tag="krot")
            v_sb = qkv.tile([P, NKT, HPG, DH + 1], F16, tag="v")
            at_sb = qkv.tile([P, 2, S], F16, tag="at")
            acc_sb = qkv.tile([P, 2, S], F32, tag="acc")

            # ---------------- phase 1: projections + rope -----------------
            with tc.tile_pool(name="p1", bufs=1) as p1:
                wq_sb = p1.tile([P, 8, 256], F16, tag="wq")
                xt_sb = p1.tile([P, 8, S], F16, tag="xt")
                wk_sb = p1.tile([P, 8, 256], F16, tag="wk")
                wv_sb = p1.tile([P, 8, 256], F16, tag="wv")
                cos_sb = p1.tile([P, S], F16, tag="cos")
                sin_sb = p1.tile([P, S], F16, tag="sin")
                st_sb = p1.tile([P, P], F16, tag="st")

                # bulk operands on the sync (HW DGE) queue, wq + x first so
                # the first projection matmuls can start ASAP
                nc.sync.dma_start(
                    out=wq_sb[:], in_=wqT.rearrange("(n p) m -> p n m", p=P)
                )
                for dt in range(8):
                    nc.sync.dma_start(
                        out=xt_sb[:, dt, :], in_=xT[P * dt:P * (dt + 1), :]
                    )
                nc.sync.dma_start(
                    out=wk_sb[:], in_=wkT.rearrange("(n p) m -> p n m", p=P)
                )
                nc.sync.dma_start(
                    out=wv_sb[:], in_=wvT.rearrange("(n p) m -> p n m", p=P)
                )
                # constants on the gpsimd (SW DGE) queue in parallel
                nc.gpsimd.dma_start(out=sin_sb[:], in_=sinT[:, :])
                nc.gpsimd.dma_start(out=cos_sb[:], in_=cosT[:, :])
                nc.gpsimd.dma_start(out=st_sb[:], in_=ST[:, :])
                nc.gpsimd.dma_start(out=tri_sb[:], in_=trimask[:, :])

                nc.gpsimd.dma_start(
                    out=wo_sb[:], in_=woT.rearrange("(n p) m -> p n m", p=P)
                )

                nc.vector.memset(v_sb[:, :, :, DH:DH + 1], 1.0)

                # Q/K projections + rope
                for w_sb, rot in ((wq_sb, qrot), (wk_sb, krot)):
                    for mt in range(2):
                        for sc in range(4):
                            ssl = bass.ts(sc, 512)
                            pp = psum.tile([P, 512], F32, tag="proj", bufs=2)
                            for dt in range(8):
                                nc.tensor.matmul(
                                    pp[:],
                                    w_sb[:, dt, P * mt:P * (mt + 1)],
                                    xt_sb[:, dt, ssl],
                                    start=(dt == 0),
                                    stop=(dt == 7),
                                )
                            t_s = p1.tile([P, 512], F16, tag="ts", bufs=3)
                            nc.vector.tensor_tensor(
                                out=t_s[:], in0=pp[:], in1=sin_sb[:, ssl],
                                op=mybir.AluOpType.mult,
                            )
                            sh = psum.tile([P, 512], F32, tag="shuf", bufs=1)
                            nc.tensor.matmul(
                                sh[:], st_sb[:], t_s[:], start=True, stop=True
                            )
                            nc.vector.tensor_tensor(
                                out=rot[:, mt, ssl], in0=pp[:],
                                in1=cos_sb[:, ssl], op=mybir.AluOpType.mult,
                            )
                            nc.vector.tensor_tensor(
                                out=rot[:, mt, ssl], in0=rot[:, mt, ssl],
                                in1=sh[:], op=mybir.AluOpType.add,
                            )

                # V projection -> fp16 V_aug (dense PE bridge into attention)
                for st in range(NKT):
                    vp = psum.tile([P, 256], F32, tag="vproj", bufs=1)
                    for dt in range(8):
                        nc.tensor.matmul(
                            vp[:],
                            xt_sb[:, dt, P * st:P * (st + 1)],
                            wv_sb[:, dt, :],
                            start=(dt == 0),
                            stop=(dt == 7),
                        )
                    nc.vector.tensor_copy(
                        out=v_sb[:, st, :, 0:DH],
                        in_=vp.rearrange("p (h d) -> p h d", h=HPG),
                    )

            # ---------------- phase 2+3: attention + output projection -----
            with tc.tile_pool(name="atmp", bufs=3) as atmp:
                eps = [
                    qkv.tile([P, EXP_TOT], F16, tag=f"expp{i}", name=f"ep{i}")
                    for i in range(2)
                ]
                piece_ctr = 0
                for h in range(HPG):
                    ep = eps[h % 2]
                    th, bs = h // 2, 64 * (h % 2)
                    qh = qrot[bs:bs + 64, th, :]
                    kh = krot[bs:bs + 64, th, :]
                    for j in range(NKT):
                        c0 = j // 4
                        r = j % 4
                        off = int(_OFF[j])
                        cs = list(range(c0, 4))
                        grps = [cs[i:i + 2] for i in range(0, len(cs), 2)]
                        for grp in grps:
                            ca = grp[0]
                            tag = "sA" if piece_ctr % 2 == 0 else "sB"
                            piece_ctr += 1
                            sp = psum.tile(
                                [P, 1024], F32, tag=tag, bufs=1, name="sp"
                            )
                            for c in grp:
                                loc = 512 * (c - ca)
                                if c == c0:
                                    nc.tensor.matmul(
                                        sp[:, loc + 128 * r:loc + 512],
                                        kh[:, P * j:P * (j + 1)],
                                        qh[:, 512 * c + 128 * r:512 * (c + 1)],
                                        start=True, stop=True,
                                    )
                                else:
                                    nc.tensor.matmul(
                                        sp[:, loc:loc + 512],
                                        kh[:, P * j:P * (j + 1)],
                                        qh[:, 512 * c:512 * (c + 1)],
                                        start=True, stop=True,
                                    )
                            # exp, skipping the sub-diagonal garbage columns
                            ls = 128 * r if ca == c0 else 0
                            qstart = 512 * ca + ls
                            w = 512 * (grp[-1] + 1) - qstart
                            eo = off + qstart - 128 * j
                            nc.scalar.activation(
                                out=ep[:, eo:eo + w],
                                in_=sp[:, ls:ls + w],
                                func=mybir.ActivationFunctionType.Exp,
                                scale=0.125,
                            )
                        nc.vector.tensor_tensor(
                            out=ep[:, off:off + P], in0=ep[:, off:off + P],
                            in1=tri_sb[:], op=mybir.AluOpType.mult,
                        )
                    # P.T @ V per query chunk (+ softmax denominator row)
                    for c in range(NQC):
                        pv = psum.tile(
                            [P, 512], F32, tag="proj", bufs=2, name="pv"
                        )
                        last_j = 4 * c + 3
                        for j in range(last_j + 1):
                            off = int(_OFF[j])
                            if j // 4 == c:
                                rr = j % 4
                                n = 512 - 128 * rr
                                nc.tensor.matmul(
                                    pv[0:DH + 1, 128 * rr:512],
                                    v_sb[:, j, h, :],
                                    ep[:, off:off + n],
                                    start=(j == 0), stop=(j == last_j),
                                )
                            else:
                                st_col = off + 512 * c - 128 * j
                                nc.tensor.matmul(
                                    pv[0:DH + 1, :],
                                    v_sb[:, j, h, :],
                                    ep[:, st_col:st_col + 512],
                                    start=(j == 0), stop=(j == last_j),
                                )
                        den = atmp.tile([1, 512], F32, tag="den")
                        nc.vector.tensor_copy(out=den[:], in_=pv[DH:DH + 1, :])
                        recip = atmp.tile([1, 512], F32, tag="recip")
                        nc.vector.reciprocal_approx_fast(out=recip[:], in_=den[:])
                        bcast = atmp.tile([64, 512], F32, tag="bcast")
                        nc.gpsimd.partition_broadcast(bcast[:], recip[:])
                        nc.vector.tensor_tensor(
                            out=at_sb[bs:bs + 64, th, 512 * c:512 * (c + 1)],
                            in0=pv[0:DH, :], in1=bcast[:],
                            op=mybir.AluOpType.mult,
                        )
                    # all-gather this head, then fold its slice of the output
                    # projection into the accumulator (runs under later heads)
                    nc.sync.dma_start(
                        out=ag_in[h, :, :], in_=at_sb[bs:bs + 64, th, :]
                    )
                    nc.gpsimd.collective_compute(
                        "AllGather",
                        mybir.AluOpType.bypass,
                        replica_groups=groups,
                        ins=[ag_in[h, :, :]],
                        outs=[ag_out[h, :, :]],
                    )
                    for sc in range(4):
                        ssl = bass.ts(sc, 512)
                        rhs = [None, None]
                        for ct in range(2):
                            rhs[ct] = agp.tile(
                                [P, 512], F16, tag="ag", name="rhs"
                            )
                            nc.sync.dma_start(
                                out=rhs[ct][:],
                                in_=ag_out[h, P * ct:P * (ct + 1), ssl],
                            )
                        for ot in range(2):
                            po = psum.tile(
                                [P, 512], F32, tag="shuf", bufs=1, name="po"
                            )
                            for ct in range(2):
                                nc.tensor.matmul(
                                    po[:],
                                    wo_sb[:, 2 * h + ct, P * ot:P * (ot + 1)],
                                    rhs[ct][:],
                                    start=(ct == 0), stop=(ct == 1),
                                )
                            if h == 0:
                                nc.vector.tensor_copy(
                                    out=acc_sb[:, ot, ssl], in_=po[:]
                                )
                            else:
                                nc.vector.tensor_tensor(
                                    out=acc_sb[:, ot, ssl],
                                    in0=acc_sb[:, ot, ssl], in1=po[:],
                                    op=mybir.AluOpType.add,
                                )
                for ot in range(2):
                    nc.sync.dma_start(
                        out=outT[P * ot:P * (ot + 1), :], in_=acc_sb[:, ot, :]
                    )

    nc.compile()
    return nc


_PROGRAM = None


def _get_program():
    global _PROGRAM
    if _PROGRAM is None:
        _PROGRAM = build_program()
    return _PROGRAM


def _host_consts(token_positions):
    pos = np.asarray(token_positions, dtype=np.float32)
    inv = (
        ROPE_THETA ** (-np.arange(0, DH, 2, dtype=np.float32) / DH)
    ).astype(np.float32)
    ang = pos[:, None] * inv[None, :]  # [S, 32]
    cos, sin = np.cos(ang), np.sin(ang)
    rows = (np.arange(P) % DH) // 2
    cosT = np.ascontiguousarray(cos.T[rows]).astype(np.float16)
    sinT = np.ascontiguousarray(sin.T[rows]).astype(np.float16)
    Smat = np.zeros((P, P), dtype=np.float32)
    idx = np.arange(0, P, 2)
    Smat[idx, idx + 1] = -1.0
    Smat[idx + 1, idx] = 1.0
    ST = np.ascontiguousarray(Smat.T).astype(np.float16)
    tri = (np.arange(P)[None, :] >= np.arange(P)[:, None]).astype(np.float16)
    return cosT, sinT, ST, tri


def _make_in_maps(x, W_q, W_k, W_v, W_o, token_positions):
    cosT, sinT, ST, tri = _host_consts(token_positions)
    x = np.asarray(x, dtype=np.float32)
    maps = []
    for core in range(NCORE):
        b, hg = core // 4, core % 4
        hsl = slice(256 * hg, 256 * (hg + 1))
        # W_o rows for this core's output columns, input dims permuted to
        # (head, group, dh) order to match the per-head AllGather layout.
        wo = np.asarray(W_o, dtype=np.float32)[hsl]          # [256, 1024]
        wo_p = wo.reshape(256, 4, HPG, DH).transpose(0, 2, 1, 3).reshape(256, D)
        maps.append(
            {
                "xT": np.ascontiguousarray(x[b].T).astype(np.float16),
                "wqT": np.ascontiguousarray(np.asarray(W_q, np.float32)[hsl].T).astype(np.float16),
                "wkT": np.ascontiguousarray(np.asarray(W_k, np.float32)[hsl].T).astype(np.float16),
                "wvT": np.ascontiguousarray(np.asarray(W_v, np.float32)[hsl].T).astype(np.float16),
                "woT": np.ascontiguousarray(wo_p.T).astype(np.float16),
                "cosT": cosT,
                "sinT": sinT,
                "ST": ST,
                "trimask": tri,
            }
        )
    return maps


def _assemble(results):
    out = np.zeros((B, S, D), dtype=np.float32)
    for core in range(NCORE):
        b, hg = core // 4, core % 4
        out[b, :, 256 * hg:256 * (hg + 1)] = results[core]["outT"].T
    return out


def _run(in_maps, trace=False):
    nc = _get_program()
    tmpdir = None
    if trace:
        import tempfile

        tmpdir = tempfile.mkdtemp(prefix="ntff_", dir="/tmp")
    res = run_bass_kernel_spmd(
        nc, in_maps, list(range(NCORE)), trace=trace, tmpdir=tmpdir
    )
    return res


def kernel(x, W_q, W_k, W_v, W_o, token_positions):
    in_maps = _make_in_maps(x, W_q, W_k, W_v, W_o, token_positions)
    res = _run(in_maps)
    return _assemble(res.results)


def _install_profile_hook():
    """The agent image's antenv lacks axon_hooks; shim it so trace=True works."""
    import sys
    import types

    try:
        from antenv.axon_hooks import get_axon_ntff_profile_hook  # noqa: F401
        return
    except ImportError:
        pass
    import antenv
    from trn_agent_boot.trn_boot import _ntff_profile_via_ctypes

    mod = types.ModuleType("antenv.axon_hooks")
    _hook = {"h": None}
    mod.set_axon_ntff_profile_hook = lambda h: _hook.__setitem__("h", h)
    mod.get_axon_ntff_profile_hook = lambda: _hook["h"]
    sys.modules["antenv.axon_hooks"] = mod
    antenv.axon_hooks = mod
    mod.set_axon_ntff_profile_hook(
        _ntff_profile_via_ctypes("/opt/axon/libaxon_pjrt.so")
    )
    import concourse.bass_utils as bu

    bu.upload_artifacts = lambda d: f"file://{d}"


def kernel_traced(x, W_q, W_k, W_v, W_o, token_positions):
    """Returns (output, exec_time_ns, trace_path)."""
    _install_profile_hook()
    in_maps = _make_in_maps(x, W_q, W_k, W_v, W_o, token_positions)
    res = _run(in_maps, trace=True)
    trace_path = None
    if res.instructions_and_trace is not None:
        trace_path = res.instructions_and_trace[1]
    return _assemble(res.results), res.exec_time_ns, trace_path
